# revision 40
# baseline (speedup 1.0000x reference)
"""Distributed causal multi-head attention for Trainium2 (8 NeuronCores).

Problem (hardcoded): x[2, 2048, 1024], 16 heads, head_dim 64, causal
softmax(QK^T/8)V then out-proj with bias. f32 in/out.

Sharding: data parallel on batch (cores 0-3 -> batch 0, 4-7 -> batch 1),
tensor parallel on heads within each group of 4 (4 heads per core).

Each core:
  - computes Q^T,K^T (head pairs packed to 128 partitions), V for its 4 heads
  - scores transposed S^T[k,q] = K Q^T so the softmax denominator comes out
    of the PE via an appended ones-column on V (no partition reductions)
  - exp without max-subtraction (scores are O(2), safe in fp32/bf16)
  - causal mask applied post-exp as a 0/1 bf16 multiply (DVE 4x mode);
    diagonal k-chunk pairs computed at reduced q-extent (512/256) to cut
    PE scores/ctx and ACT exp work ~12%
  - ROW-parallel out-proj: each core computes partial out[1024 oc, q] over
    its OWN 256 ctx rows (+ bo/4), then a ReduceScatter(add) over the
    4-core group scatters q-slices. Host reassembles q-slices per batch.
    This replaces the ctx AllGather: the collective is billed on its
    (4x smaller) output and RS0 overlaps pair-1 attention.

All matmuls bf16 (fp32 PSUM accumulation).
"""

import numpy as np
import ml_dtypes

from concourse import bass, bacc, mybir
from concourse import tile
from concourse.bass_utils import run_bass_kernel_spmd

BF16 = mybir.dt.bfloat16
F32 = mybir.dt.float32
Act = mybir.ActivationFunctionType

B, S, D = 2, 2048, 1024
H, HD = 16, 64
NCORES = 8
GROUP = 4            # cores per batch group
HPC = H // GROUP     # 4 heads per core
CW = HPC * HD        # 256 ctx rows per core
QC = 512             # q-chunk width
KC = 128             # k-chunk width
NQ = S // QC         # 4
NKC = S // KC        # 16
DCH = D // 128       # 8 contraction chunks of 128
OCT = D // 128       # 8 out-proj column tiles
# RS regions (processing order 1,2,3,0): region 0 = {j1,j2} pieces 256 at
# q=512+rank*256; region 1 = {j3} pieces 128 at 1536+rank*128; region 2 =
# {j0} pieces 128 at rank*128
PW = (256, 128, 128)
RQ0 = (512, 1536, 0)

_CACHE = {}


def _build_bass():
    nc = bacc.Bacc(
        "TRN2", target_bir_lowering=False, debug=False, num_devices=NCORES
    )
    _ccs = []
    _rds = []

    # per-core external inputs (same shapes on every core: SPMD)
    xT = nc.declare_dram_parameter("xT", [D, S], BF16, isOutput=False)
    wq = nc.declare_dram_parameter("wq", [D, CW], BF16, isOutput=False)
    wk = nc.declare_dram_parameter("wk", [D, CW], BF16, isOutput=False)
    wv = nc.declare_dram_parameter("wv", [D, CW], BF16, isOutput=False)
    wo = nc.declare_dram_parameter("wo", [CW, D], BF16, isOutput=False)
    boq = nc.declare_dram_parameter("boq", [128, OCT, 1], F32, isOutput=False)
    msk = nc.declare_dram_parameter("msk", [128, 4, QC], BF16, isOutput=False)
    vones = nc.declare_dram_parameter("vones", [128, NKC, HPC, 1], BF16, isOutput=False)
    selv = nc.declare_dram_parameter("selv", [1, 2, 128], BF16, isOutput=False)
    # ReduceScatter piece outputs: region r piece = q QOFF[r]+rank*PW[r]+
    po_out = [nc.declare_dram_parameter(f"po{h}", [D, w], BF16, isOutput=True)
              for h, w in enumerate(PW)]

    with tile.TileContext(nc) as tc:
        with tc.tile_pool(name="dram", bufs=1, space="DRAM") as dram:
            rs_in = [dram.tile([GROUP, D, w], BF16, name=f"rs_in{h}")
                     for h, w in enumerate(PW)]
            rs_out = [dram.tile([D, w], BF16, name=f"rs_out{h}")
                      for h, w in enumerate(PW)]

            with tc.tile_pool(name="persist", bufs=1) as pp:
                wq_sb = pp.tile([128, DCH, CW], BF16, tag="wq_sb")
                wk_sb = pp.tile([128, DCH, CW], BF16, tag="wk_sb")
                wv_sb = pp.tile([128, DCH, CW], BF16, tag="wv_sb")
                wo_sb = pp.tile([128, CW // 128, D], BF16, tag="wo_sb")
                boq_sb = pp.tile([128, OCT, 1], F32, tag="boq_sb")
                msk_sb = pp.tile([128, 4, QC], BF16, tag="msk_sb")
                selv_sb = pp.tile([65, 2, 128], BF16, tag="selv_sb")
                qT_sb = pp.tile([128, 2, S], BF16, tag="qT_sb")
                kT_sb = pp.tile([128, 2, S], BF16, tag="kT_sb")
                v_aug = pp.tile([128, NKC, HPC, HD + 1], BF16, tag="v_aug")
                xT_sb = pp.tile([128, DCH, S], BF16, tag="xT_sb")

                # ---- input DMAs: few, large, ordered for earliest compute ----
                def ld(dst_ap, src_ap):
                    nc.sync.dma_start(dst_ap, src_ap)

                ld(wq_sb[:], wq.rearrange("(c p) w -> p c w", p=128))
                ld(xT_sb[:, :, 0:QC // 2],
                   xT[:, 0:QC // 2].rearrange("(c p) q -> p c q", p=128))
                ld(wk_sb[:], wk.rearrange("(c p) w -> p c w", p=128))
                ld(xT_sb[:, :, QC // 2:QC],
                   xT[:, QC // 2:QC].rearrange("(c p) q -> p c q", p=128))
                ld(xT_sb[:, :, QC:2 * QC],
                   xT[:, QC:2 * QC].rearrange("(c p) q -> p c q", p=128))
                ld(wv_sb[:], wv.rearrange("(c p) w -> p c w", p=128))
                ld(xT_sb[:, :, 2 * QC:3 * QC],
                   xT[:, 2 * QC:3 * QC].rearrange("(c p) q -> p c q", p=128))
                ld(msk_sb[:], msk[:])
                ld(v_aug[:, :, :, HD:HD + 1], vones[:])
                ld(selv_sb[64:65, :, :], selv[:])
                ld(xT_sb[:, :, 3 * QC:4 * QC],
                   xT[:, 3 * QC:4 * QC].rearrange("(c p) q -> p c q", p=128))
                ld(wo_sb[:], wo.rearrange("(c p) w -> p c w", p=128))
                ld(boq_sb[:], boq[:])

                with tc.tile_pool(name="sc_ps", bufs=2, space="PSUM") as scp, \
                     tc.tile_pool(name="ct_ps", bufs=2, space="PSUM") as ctp, \
                     tc.tile_pool(name="o_ps", bufs=2, space="PSUM") as outp, \
                     tc.tile_pool(name="es_pool", bufs=18) as esp, \
                     tc.tile_pool(name="stg_pool", bufs=8) as stgp, \
                     tc.tile_pool(name="ctxn_pool", bufs=8) as cxp, \
                     tc.tile_pool(name="po_pool", bufs=2) as pop, \
                     tc.tile_pool(name="nrm", bufs=3) as nrmp, \
                     tc.tile_pool(name="misc", bufs=1) as miscp:

                    ctxn = [[None] * NQ, [None] * NQ]

                    # ---- PE filler queue: self-contained items (alloc, use
                    # and release one outp PSUM bank each) popped between
                    # attention chunk iterations so PE never idles while ACT
                    # digests exps, and ACT never idles during projections ----
                    fill_hi = []   # po items: drained first (unblock the RS)
                    fill_lo = []   # proj items: safe to run early
                    _fp = [0, 0]

                    def pop_fill(budget, jmax=NQ):
                        spent = 0.0
                        while spent < budget:
                            if _fp[0] < len(fill_hi):
                                cost, fn = fill_hi[_fp[0]]
                                _fp[0] += 1
                            elif _fp[1] < len(fill_lo) and \
                                    fill_lo[_fp[1]][2] <= jmax:
                                cost, fn = fill_lo[_fp[1]][:2]
                                _fp[1] += 1
                            else:
                                return
                            fn()
                            spent += cost

                    def drain_lo(idx):
                        while _fp[1] < idx:
                            fn = fill_lo[_fp[1]][1]
                            _fp[1] += 1
                            fn()

                    def drain_hi():
                        while _fp[0] < len(fill_hi):
                            cost, fn = fill_hi[_fp[0]]
                            _fp[0] += 1
                            fn()

                    def proj_qk_item(pair, j, w_sb, dst, q0=0, qw=QC):
                        def run():
                            qs = slice(j * QC + q0, j * QC + q0 + qw)
                            ps = outp.tile([128, QC], F32, tag="ops")
                            for c in range(DCH):
                                nc.tensor.matmul(
                                    ps[:, 0:qw],
                                    w_sb[:, c, pair * 128:(pair + 1) * 128],
                                    xT_sb[:, c, qs],
                                    start=(c == 0),
                                    stop=(c == DCH - 1),
                                )
                            nc.vector.tensor_copy(dst[:, pair, qs], ps[:, 0:qw])
                        return (1.9 * qw / QC, run)

                    def proj_v_item(pair, t):
                        def run():
                            ps = outp.tile([128, QC], F32, tag="ops")
                            for c in range(DCH):
                                nc.tensor.matmul(
                                    ps[:, 0:128],
                                    xT_sb[:, c, t * 128:(t + 1) * 128],
                                    wv_sb[:, c, pair * 128:(pair + 1) * 128],
                                    start=(c == 0),
                                    stop=(c == DCH - 1),
                                )
                            nc.vector.tensor_copy(
                                v_aug[:, t, 2 * pair:2 * pair + 2, 0:HD],
                                ps[:, 0:128].rearrange("p (h w) -> p h w", h=2),
                            )
                        return (0.8, run)

                    def attn2(pair, j, mid_mark=None, step=0):
                        mid_i = max(0, 2 * j - 2)
                        # chunk-level software pipeline across the head pair:
                        # PE emits scores(A,i), scores(B,i), then ctx for
                        # chunk-pair i-1 of both heads, so the PE never parks
                        # on the scores->exp->mask chain and ACT stays fed.
                        ha, hb = 2 * pair, 2 * pair + 1
                        nkc = (j + 1) * 4
                        npr = nkc // 2
                        es = {ha: [], hb: []}
                        offs = []

                        def scores_pair(h, i, qa):
                            row = (h % 2) * 64
                            st = scp.tile([128, 2, QC], F32, tag="st")
                            for k in range(2):
                                c = 2 * i + k
                                nc.tensor.matmul(
                                    st[:, k, qa:],
                                    kT_sb[row:row + 64, pair, c * KC:(c + 1) * KC],
                                    qT_sb[row:row + 64, pair, j * QC + qa:(j + 1) * QC],
                                    start=True, stop=True,
                                )
                            e = esp.tile([128, 2, QC], BF16, tag="es")
                            nc.scalar.activation(e[:, :, qa:], st[:, :, qa:],
                                                 Act.Exp, scale=0.125)
                            if i >= npr - 2:
                                m0 = 2 * i - (nkc - 4)
                                nc.vector.tensor_mul(
                                    e[:, :, qa:], e[:, :, qa:],
                                    msk_sb[:, m0:m0 + 2, qa:],
                                )
                            es[h].append(e)

                        def ctx_pair(ct, h, i):
                            qa = offs[i]
                            for k in range(2):
                                c = 2 * i + k
                                nc.tensor.matmul(
                                    ct[:, qa:],
                                    v_aug[:, c, h, :],
                                    es[h][i][:, k, qa:],
                                    start=(c == 0),
                                    stop=(c == nkc - 1),
                                )

                        ctA = ctp.tile([HD + 1, QC], F32, tag="ct")
                        ctB = ctp.tile([HD + 1, QC], F32, tag="ct")
                        for i in range(npr):
                            qa = 256 if i == npr - 1 else 0
                            offs.append(qa)
                            scores_pair(ha, i, qa)
                            scores_pair(hb, i, qa)
                            if i >= 2:
                                ctx_pair(ctA, ha, i - 2)
                                ctx_pair(ctB, hb, i - 2)
                            if i == mid_i and mid_mark is not None:
                                drain_lo(mid_mark)
                            pop_fill(0.8, step + 1)
                        ctx_pair(ctA, ha, npr - 2)
                        ctx_pair(ctB, hb, npr - 2)
                        ctx_pair(ctA, ha, npr - 1)
                        ctx_pair(ctB, hb, npr - 1)
                        # ctx rows + den row in one copy (bf16 den: ~0.2% noise)
                        sa = stgp.tile([HD + 1, QC], BF16, tag="stg")
                        nc.vector.tensor_copy(sa[:], ctA[:])
                        sb = stgp.tile([HD + 1, QC], BF16, tag="stg")
                        nc.vector.tensor_copy(sb[:], ctB[:])
                        norm(pair, j, sa, sb)

                    def norm(pair, j, stg_even, stg_odd):
                        bc = outp.tile([128, QC], F32, tag="ops")
                        nc.tensor.matmul(bc[:], selv_sb[64:65, 0, :],
                                         stg_even[HD:HD + 1, :],
                                         start=True, stop=False)
                        nc.tensor.matmul(bc[:], selv_sb[64:65, 1, :],
                                         stg_odd[HD:HD + 1, :],
                                         start=False, stop=True)
                        rb = nrmp.tile([64, 2, QC], BF16, tag="rb")
                        with nc.allow_low_precision(reason="1/den in bf16: ~0.2% fro"):
                            nc.vector.reciprocal(rb[:, 0, :], bc[0:64, :])
                            nc.vector.reciprocal(rb[:, 1, :], bc[64:128, :])
                        cx = cxp.tile([128, QC], BF16, tag="ctxn")
                        nc.vector.tensor_mul(cx[0:64, :], stg_even[0:HD, :], rb[:, 0, :])
                        nc.vector.tensor_mul(cx[64:128, :], stg_odd[0:HD, :], rb[:, 1, :])
                        ctxn[pair][j] = cx

                    def po_items(j):
                        posb = pop.tile([128, OCT, QC], BF16, tag="posb")
                        tail = (j == 0)   # j=0 is processed LAST

                        def mk_o(o):
                            def run():
                                ps = outp.tile([128, QC], F32, tag="ops")
                                nc.tensor.matmul(
                                    ps[:], wo_sb[:, 0, o * 128:(o + 1) * 128],
                                    ctxn[0][j][:], start=True, stop=False)
                                nc.tensor.matmul(
                                    ps[:], wo_sb[:, 1, o * 128:(o + 1) * 128],
                                    ctxn[1][j][:], start=False, stop=True)
                                with nc.allow_low_precision(reason="bf16 RS partials"):
                                    if tail and o % 2 == 1:
                                        # ACT is idle in the tail: alternate so
                                        # the po stream isn't DVE-paced
                                        nc.scalar.activation(
                                            posb[:, o, :], ps[:], Act.Identity,
                                            bias=boq_sb[:, o, :])
                                    else:
                                        nc.vector.tensor_scalar_add(
                                            posb[:, o, :], ps[:], boq_sb[:, o, :])
                                if j in (3, 0):
                                    # 128-wide-piece regions: per-o-tile scatter
                                    # fires immediately so the RS input is
                                    # ready as soon as the last add lands
                                    r = 1 if j == 3 else 2
                                    nc.sync.dma_start(
                                        rs_in[r][:, o * 128:(o + 1) * 128, :]
                                        .rearrange("r p q -> p r q"),
                                        posb[:, o, :].rearrange(
                                            "p (r q) -> p r q", r=GROUP),
                                    )
                            return (0.6, run)

                        def dma_run():
                            # region 0 ({j1,j2}, 256-wide pieces): j-chunks
                            # align to exactly two slabs
                            for s2 in range(2):
                                slab = 2 * (j - 1) + s2
                                nc.sync.dma_start(
                                    rs_in[0][slab, :, :].rearrange(
                                        "(o p) q -> p o q", p=128),
                                    posb[:, :, s2 * 256:(s2 + 1) * 256],
                                )

                        for o in range(OCT):
                            fill_hi.append(mk_o(o))
                        if j in (1, 2):
                            fill_hi.append((0.1, dma_run))
                        if j == 2:
                            fill_hi.append((0.1, lambda: rs(0)))
                        if j == 3:
                            fill_hi.append((0.1, lambda: rs(1)))

                    def rs(r):
                        _ccs.append(nc.gpsimd.collective_compute(
                            "ReduceScatter",
                            mybir.AluOpType.add,
                            replica_groups=[[0, 1, 2, 3], [4, 5, 6, 7]],
                            ins=[rs_in[r][:].opt()],
                            outs=[rs_out[r][:].opt()],
                        ))

                    # ---- schedule: q-chunks processed in order (1,2,3,0) so
                    # the LAST attention chunk is the cheapest (j=0, ~4x less
                    # exp than j=3) -> minimal ACT-paced tail before the final
                    # RS; pairs interleaved per chunk; projections and
                    # out-proj partials drain through the filler queue ----
                    PORDER = (1, 2, 3, 0)
                    marks_pre = {}
                    marks_full = {}

                    def lo(item, step):
                        fill_lo.append((item[0], item[1], step))
                    for step, j in enumerate(PORDER):
                        for pair in (0, 1):
                            if j == 1:
                                # first processed chunk also needs k-tile 0
                                lo(proj_qk_item(pair, 1, wq_sb, qT_sb,
                                                0, QC // 2), step)
                                lo(proj_qk_item(pair, 1, wq_sb, qT_sb,
                                                QC // 2, QC // 2), step)
                                lo(proj_qk_item(pair, 0, wk_sb, kT_sb), step)
                                marks_pre[(pair, j)] = len(fill_lo)
                                lo(proj_qk_item(pair, 1, wk_sb, kT_sb), step)
                                for t in range(0, 8):
                                    lo(proj_v_item(pair, t), step)
                            elif j == 0:
                                lo(proj_qk_item(pair, 0, wq_sb, qT_sb), step)
                                marks_pre[(pair, j)] = len(fill_lo)
                            else:
                                lo(proj_qk_item(pair, j, wq_sb, qT_sb), step)
                                marks_pre[(pair, j)] = len(fill_lo)
                                lo(proj_qk_item(pair, j, wk_sb, kT_sb), step)
                                for t in range(4 * j, 4 * j + 4):
                                    lo(proj_v_item(pair, t), step)
                            marks_full[(pair, j)] = len(fill_lo)
                    for step, j in enumerate(PORDER):
                        for pair in (0, 1):
                            drain_lo(marks_pre[(pair, j)])
                            attn2(pair, j, mid_mark=marks_full[(pair, j)],
                                  step=step)
                        po_items(j)
                    drain_hi()
                    drain_lo(len(fill_lo))
                    rs(2)
                    # piece readbacks last: their sem waits must not head-of-
                    # line block the SP queue ahead of the scatter DMAs
                    for r in range(3):
                        _rds.append((nc.sync.dma_start(po_out[r][:], rs_out[r][:]), r))

    upd = _ccs[0].ins.sync_info.on_update[0]
    cc_done_sem = bass.SemaphoreHandle(upd.ant_name, upd.id)
    for rd, h in _rds:
        rd.wait_op(cc_done_sem, h + 1, "sem-ge", check=False)
    nc.compile()
    return nc


def _causal_mask():
    # msk[kp, m, qf] = 1 where (m*128 + kp) <= qf else 0  (keep k <= q)
    kp = np.arange(128)[:, None, None]
    m = np.arange(4)[None, :, None]
    qf = np.arange(QC)[None, None, :]
    return (m * 128 + kp <= qf).astype(ml_dtypes.bfloat16)


def _in_maps(x, Wq, Wk, Wv, Wo, bo):
    bf = ml_dtypes.bfloat16
    msk = _causal_mask()
    selv = np.zeros((1, 2, 128), dtype=bf)
    selv[0, 0, 0:64] = 1.0
    selv[0, 1, 64:128] = 1.0
    boq = (bo.reshape(OCT, 128).T / GROUP).astype(np.float32)[:, :, None]
    xT = [np.ascontiguousarray(x[b].T).astype(bf) for b in range(B)]
    maps = []
    for c in range(NCORES):
        b, g = c // GROUP, c % GROUP
        cs = slice(g * CW, (g + 1) * CW)
        maps.append({
            "xT": xT[b],
            "wq": np.ascontiguousarray(Wq[:, cs]).astype(bf),
            "wk": np.ascontiguousarray(Wk[:, cs]).astype(bf),
            "wv": np.ascontiguousarray(Wv[:, cs]).astype(bf),
            "wo": np.ascontiguousarray(Wo[cs, :]).astype(bf),
            "boq": boq,
            "msk": msk,
            "vones": np.ones((128, NKC, HPC, 1), dtype=bf),
            "selv": selv,
        })
    return maps


def kernel(x, Wq, Wk, Wv, Wo, bo, _trace=False):
    x = np.asarray(x, dtype=np.float32)
    Wq, Wk, Wv, Wo, bo = (np.asarray(a, dtype=np.float32) for a in (Wq, Wk, Wv, Wo, bo))
    if "nc" not in _CACHE:
        _CACHE["nc"] = _build_bass()
    nc = _CACHE["nc"]
    res = run_bass_kernel_spmd(
        nc, _in_maps(x, Wq, Wk, Wv, Wo, bo), list(range(NCORES)), trace=_trace
    )
    out = np.zeros((B, S, D), dtype=np.float32)
    for c in range(NCORES):
        b, g = c // GROUP, c % GROUP
        for r in range(3):
            piece = np.asarray(res.results[c][f"po{r}"]).astype(np.float32)
            q0 = RQ0[r] + g * PW[r]
            out[b, q0:q0 + PW[r], :] = piece.T
    if _trace:
        return out, res
    return out


# revision 41
# speedup vs baseline: 1.0707x; 1.0707x over previous
"""Distributed causal multi-head attention for Trainium2 (8 NeuronCores).

Problem (hardcoded): x[2, 2048, 1024], 16 heads, head_dim 64, causal
softmax(QK^T/8)V then out-proj with bias. f32 in/out.

Sharding: data parallel on batch (cores 0-3 -> batch 0, 4-7 -> batch 1),
tensor parallel on heads within each group of 4 (4 heads per core).

Each core:
  - computes Q^T,K^T (head pairs packed to 128 partitions), V for its 4 heads
  - scores transposed S^T[k,q] = K Q^T so the softmax denominator comes out
    of the PE via an appended ones-column on V (no partition reductions)
  - exp without max-subtraction (scores are O(2), safe in fp32/bf16)
  - causal mask applied post-exp as a 0/1 bf16 multiply (DVE 4x mode);
    diagonal k-chunk pairs computed at reduced q-extent (512/256) to cut
    PE scores/ctx and ACT exp work ~12%
  - ROW-parallel out-proj: each core computes partial out[1024 oc, q] over
    its OWN 256 ctx rows (+ bo/4), then a ReduceScatter(add) over the
    4-core group scatters q-slices. Host reassembles q-slices per batch.
    This replaces the ctx AllGather: the collective is billed on its
    (4x smaller) output and RS0 overlaps pair-1 attention.

All matmuls bf16 (fp32 PSUM accumulation).
"""

import numpy as np
import ml_dtypes

from concourse import bass, bacc, mybir
from concourse import tile
from concourse.bass_utils import run_bass_kernel_spmd

BF16 = mybir.dt.bfloat16
F32 = mybir.dt.float32
Act = mybir.ActivationFunctionType

B, S, D = 2, 2048, 1024
H, HD = 16, 64
NCORES = 8
GROUP = 4            # cores per batch group
HPC = H // GROUP     # 4 heads per core
CW = HPC * HD        # 256 ctx rows per core
QC = 512             # q-chunk width
KC = 128             # k-chunk width
NQ = S // QC         # 4
NKC = S // KC        # 16
DCH = D // 128       # 8 contraction chunks of 128
OCT = D // 128       # 8 out-proj column tiles
# RS regions (processing order 0,1,3,2): region 0 = {j0,j1} pieces 256 at
# q=rank*256; region 1 = {j3} pieces 128 at 1536+rank*128; region 2 =
# {j2} pieces 128 at 1024+rank*128 (the tail: j2 is processed last and its
# ~24us of attention covers region 1's RS, while being cheaper than j3)
PW = (256, 128, 128)
RQ0 = (0, 1536, 1024)

_CACHE = {}


def _build_bass():
    nc = bacc.Bacc(
        "TRN2", target_bir_lowering=False, debug=False, num_devices=NCORES
    )
    _ccs = []
    _rds = []

    # per-core external inputs (same shapes on every core: SPMD)
    xT = nc.declare_dram_parameter("xT", [D, S], BF16, isOutput=False)
    wq = nc.declare_dram_parameter("wq", [D, CW], BF16, isOutput=False)
    wk = nc.declare_dram_parameter("wk", [D, CW], BF16, isOutput=False)
    wv = nc.declare_dram_parameter("wv", [D, CW], BF16, isOutput=False)
    wo = nc.declare_dram_parameter("wo", [CW, D], BF16, isOutput=False)
    boq = nc.declare_dram_parameter("boq", [128, OCT, 1], F32, isOutput=False)
    msk = nc.declare_dram_parameter("msk", [128, 4, QC], BF16, isOutput=False)
    vones = nc.declare_dram_parameter("vones", [128, NKC, HPC, 1], BF16, isOutput=False)
    selv = nc.declare_dram_parameter("selv", [1, 2, 128], BF16, isOutput=False)
    # ReduceScatter piece outputs: region r piece = q QOFF[r]+rank*PW[r]+
    po_out = [nc.declare_dram_parameter(f"po{h}", [D, w], BF16, isOutput=True)
              for h, w in enumerate(PW)]

    with tile.TileContext(nc) as tc:
        with tc.tile_pool(name="dram", bufs=1, space="DRAM") as dram:
            rs_in = [dram.tile([GROUP, D, w], BF16, name=f"rs_in{h}")
                     for h, w in enumerate(PW)]
            rs_out = [dram.tile([D, w], BF16, name=f"rs_out{h}")
                      for h, w in enumerate(PW)]

            with tc.tile_pool(name="persist", bufs=1) as pp:
                wq_sb = pp.tile([128, DCH, CW], BF16, tag="wq_sb")
                wk_sb = pp.tile([128, DCH, CW], BF16, tag="wk_sb")
                wv_sb = pp.tile([128, DCH, CW], BF16, tag="wv_sb")
                wo_sb = pp.tile([128, CW // 128, D], BF16, tag="wo_sb")
                boq_sb = pp.tile([128, OCT, 1], F32, tag="boq_sb")
                msk_sb = pp.tile([128, 4, QC], BF16, tag="msk_sb")
                selv_sb = pp.tile([65, 2, 128], BF16, tag="selv_sb")
                qT_sb = pp.tile([128, 2, S], BF16, tag="qT_sb")
                kT_sb = pp.tile([128, 2, S], BF16, tag="kT_sb")
                v_aug = pp.tile([128, NKC, HPC, HD + 1], BF16, tag="v_aug")
                xT_sb = pp.tile([128, DCH, S], BF16, tag="xT_sb")

                # ---- input DMAs: few, large, ordered for earliest compute ----
                def ld(dst_ap, src_ap):
                    nc.sync.dma_start(dst_ap, src_ap)

                ld(wq_sb[:], wq.rearrange("(c p) w -> p c w", p=128))
                ld(xT_sb[:, :, 0:QC // 2],
                   xT[:, 0:QC // 2].rearrange("(c p) q -> p c q", p=128))
                ld(wk_sb[:], wk.rearrange("(c p) w -> p c w", p=128))
                ld(xT_sb[:, :, QC // 2:QC],
                   xT[:, QC // 2:QC].rearrange("(c p) q -> p c q", p=128))
                ld(xT_sb[:, :, QC:2 * QC],
                   xT[:, QC:2 * QC].rearrange("(c p) q -> p c q", p=128))
                ld(wv_sb[:], wv.rearrange("(c p) w -> p c w", p=128))
                ld(xT_sb[:, :, 2 * QC:3 * QC],
                   xT[:, 2 * QC:3 * QC].rearrange("(c p) q -> p c q", p=128))
                ld(msk_sb[:], msk[:])
                ld(v_aug[:, :, :, HD:HD + 1], vones[:])
                ld(selv_sb[64:65, :, :], selv[:])
                ld(xT_sb[:, :, 3 * QC:4 * QC],
                   xT[:, 3 * QC:4 * QC].rearrange("(c p) q -> p c q", p=128))
                ld(wo_sb[:], wo.rearrange("(c p) w -> p c w", p=128))
                ld(boq_sb[:], boq[:])

                with tc.tile_pool(name="sc_ps", bufs=2, space="PSUM") as scp, \
                     tc.tile_pool(name="ct_ps", bufs=2, space="PSUM") as ctp, \
                     tc.tile_pool(name="o_ps", bufs=2, space="PSUM") as outp, \
                     tc.tile_pool(name="es_pool", bufs=18) as esp, \
                     tc.tile_pool(name="stg_pool", bufs=8) as stgp, \
                     tc.tile_pool(name="ctxn_pool", bufs=8) as cxp, \
                     tc.tile_pool(name="po_pool", bufs=2) as pop, \
                     tc.tile_pool(name="nrm", bufs=3) as nrmp, \
                     tc.tile_pool(name="misc", bufs=1) as miscp:

                    ctxn = [[None] * NQ, [None] * NQ]

                    # ---- PE filler queue: self-contained items (alloc, use
                    # and release one outp PSUM bank each) popped between
                    # attention chunk iterations so PE never idles while ACT
                    # digests exps, and ACT never idles during projections ----
                    fill_hi = []   # po items: drained first (unblock the RS)
                    fill_lo = []   # proj items: safe to run early
                    _fp = [0, 0]

                    def pop_fill(budget, jmax=NQ):
                        spent = 0.0
                        while spent < budget:
                            if _fp[0] < len(fill_hi):
                                cost, fn = fill_hi[_fp[0]]
                                _fp[0] += 1
                            elif _fp[1] < len(fill_lo) and \
                                    fill_lo[_fp[1]][2] <= jmax:
                                cost, fn = fill_lo[_fp[1]][:2]
                                _fp[1] += 1
                            else:
                                return
                            fn()
                            spent += cost

                    def drain_lo(idx):
                        while _fp[1] < idx:
                            fn = fill_lo[_fp[1]][1]
                            _fp[1] += 1
                            fn()

                    def drain_hi():
                        while _fp[0] < len(fill_hi):
                            cost, fn = fill_hi[_fp[0]]
                            _fp[0] += 1
                            fn()

                    def proj_qk_item(pair, j, w_sb, dst, q0=0, qw=QC):
                        def run():
                            qs = slice(j * QC + q0, j * QC + q0 + qw)
                            ps = outp.tile([128, QC], F32, tag="ops")
                            for c in range(DCH):
                                nc.tensor.matmul(
                                    ps[:, 0:qw],
                                    w_sb[:, c, pair * 128:(pair + 1) * 128],
                                    xT_sb[:, c, qs],
                                    start=(c == 0),
                                    stop=(c == DCH - 1),
                                )
                            nc.vector.tensor_copy(dst[:, pair, qs], ps[:, 0:qw])
                        return (1.9 * qw / QC, run)

                    def proj_v_item(pair, t):
                        def run():
                            ps = outp.tile([128, QC], F32, tag="ops")
                            for c in range(DCH):
                                nc.tensor.matmul(
                                    ps[:, 0:128],
                                    xT_sb[:, c, t * 128:(t + 1) * 128],
                                    wv_sb[:, c, pair * 128:(pair + 1) * 128],
                                    start=(c == 0),
                                    stop=(c == DCH - 1),
                                )
                            nc.vector.tensor_copy(
                                v_aug[:, t, 2 * pair:2 * pair + 2, 0:HD],
                                ps[:, 0:128].rearrange("p (h w) -> p h w", h=2),
                            )
                        return (0.8, run)

                    def attn2(pair, j, drains=(), step=0):
                        # chunk-level software pipeline across the head pair:
                        # PE emits scores(A,i), scores(B,i), then ctx for
                        # chunk-pair i-1 of both heads, so the PE never parks
                        # on the scores->exp->mask chain and ACT stays fed.
                        ha, hb = 2 * pair, 2 * pair + 1
                        nkc = (j + 1) * 4
                        npr = nkc // 2
                        es = {ha: [], hb: []}
                        offs = []

                        def scores_pair(h, i, qa):
                            row = (h % 2) * 64
                            st = scp.tile([128, 2, QC], F32, tag="st")
                            for k in range(2):
                                c = 2 * i + k
                                nc.tensor.matmul(
                                    st[:, k, qa:],
                                    kT_sb[row:row + 64, pair, c * KC:(c + 1) * KC],
                                    qT_sb[row:row + 64, pair, j * QC + qa:(j + 1) * QC],
                                    start=True, stop=True,
                                )
                            e = esp.tile([128, 2, QC], BF16, tag="es")
                            nc.scalar.activation(e[:, :, qa:], st[:, :, qa:],
                                                 Act.Exp, scale=0.125)
                            if i >= npr - 2:
                                m0 = 2 * i - (nkc - 4)
                                nc.vector.tensor_mul(
                                    e[:, :, qa:], e[:, :, qa:],
                                    msk_sb[:, m0:m0 + 2, qa:],
                                )
                            es[h].append(e)

                        def ctx_pair(ct, h, i):
                            qa = offs[i]
                            for k in range(2):
                                c = 2 * i + k
                                nc.tensor.matmul(
                                    ct[:, qa:],
                                    v_aug[:, c, h, :],
                                    es[h][i][:, k, qa:],
                                    start=(c == 0),
                                    stop=(c == nkc - 1),
                                )

                        ctA = ctp.tile([HD + 1, QC], F32, tag="ct")
                        ctB = ctp.tile([HD + 1, QC], F32, tag="ct")
                        for i in range(npr):
                            qa = 256 if i == npr - 1 else 0
                            offs.append(qa)
                            scores_pair(ha, i, qa)
                            scores_pair(hb, i, qa)
                            if i >= 2:
                                ctx_pair(ctA, ha, i - 2)
                                ctx_pair(ctB, hb, i - 2)
                            for di, dmark in drains:
                                if i == di:
                                    drain_lo(dmark)
                            pop_fill(0.8, step + 1)
                        ctx_pair(ctA, ha, npr - 2)
                        ctx_pair(ctB, hb, npr - 2)
                        ctx_pair(ctA, ha, npr - 1)
                        ctx_pair(ctB, hb, npr - 1)
                        # ctx rows + den row in one copy (bf16 den: ~0.2% noise)
                        sa = stgp.tile([HD + 1, QC], BF16, tag="stg")
                        nc.vector.tensor_copy(sa[:], ctA[:])
                        sb = stgp.tile([HD + 1, QC], BF16, tag="stg")
                        nc.vector.tensor_copy(sb[:], ctB[:])
                        norm(pair, j, sa, sb)

                    def norm(pair, j, stg_even, stg_odd):
                        bc = outp.tile([128, QC], F32, tag="ops")
                        nc.tensor.matmul(bc[:], selv_sb[64:65, 0, :],
                                         stg_even[HD:HD + 1, :],
                                         start=True, stop=False)
                        nc.tensor.matmul(bc[:], selv_sb[64:65, 1, :],
                                         stg_odd[HD:HD + 1, :],
                                         start=False, stop=True)
                        rb = nrmp.tile([64, 2, QC], BF16, tag="rb")
                        with nc.allow_low_precision(reason="1/den in bf16: ~0.2% fro"):
                            nc.vector.reciprocal(rb[:, 0, :], bc[0:64, :])
                            nc.vector.reciprocal(rb[:, 1, :], bc[64:128, :])
                        cx = cxp.tile([128, QC], BF16, tag="ctxn")
                        nc.vector.tensor_mul(cx[0:64, :], stg_even[0:HD, :], rb[:, 0, :])
                        nc.vector.tensor_mul(cx[64:128, :], stg_odd[0:HD, :], rb[:, 1, :])
                        ctxn[pair][j] = cx

                    def po_items(j):
                        posb = pop.tile([128, OCT, QC], BF16, tag="posb")
                        tail = (j == 2)   # j=2 is processed LAST

                        def mk_o(o):
                            def run():
                                ps = outp.tile([128, QC], F32, tag="ops")
                                nc.tensor.matmul(
                                    ps[:], wo_sb[:, 0, o * 128:(o + 1) * 128],
                                    ctxn[0][j][:], start=True, stop=False)
                                nc.tensor.matmul(
                                    ps[:], wo_sb[:, 1, o * 128:(o + 1) * 128],
                                    ctxn[1][j][:], start=False, stop=True)
                                with nc.allow_low_precision(reason="bf16 RS partials"):
                                    if tail and o % 2 == 1:
                                        # ACT is idle in the tail: alternate so
                                        # the po stream isn't DVE-paced
                                        nc.scalar.activation(
                                            posb[:, o, :], ps[:], Act.Identity,
                                            bias=boq_sb[:, o, :])
                                    else:
                                        nc.vector.tensor_scalar_add(
                                            posb[:, o, :], ps[:], boq_sb[:, o, :])
                                if j in (3, 2):
                                    # 128-wide-piece regions: per-o-tile scatter
                                    # fires immediately so the RS input is
                                    # ready as soon as the last add lands
                                    r = 1 if j == 3 else 2
                                    nc.sync.dma_start(
                                        rs_in[r][:, o * 128:(o + 1) * 128, :]
                                        .rearrange("r p q -> p r q"),
                                        posb[:, o, :].rearrange(
                                            "p (r q) -> p r q", r=GROUP),
                                    )
                            return (0.6, run)

                        def dma_run():
                            # region 0 ({j0,j1}, 256-wide pieces): j-chunks
                            # align to exactly two slabs
                            for s2 in range(2):
                                slab = 2 * j + s2
                                nc.sync.dma_start(
                                    rs_in[0][slab, :, :].rearrange(
                                        "(o p) q -> p o q", p=128),
                                    posb[:, :, s2 * 256:(s2 + 1) * 256],
                                )

                        for o in range(OCT):
                            fill_hi.append(mk_o(o))
                        if j in (0, 1):
                            fill_hi.append((0.1, dma_run))
                        if j == 1:
                            fill_hi.append((0.1, lambda: rs(0)))
                        if j == 3:
                            fill_hi.append((0.1, lambda: rs(1)))

                    def rs(r):
                        _ccs.append(nc.gpsimd.collective_compute(
                            "ReduceScatter",
                            mybir.AluOpType.add,
                            replica_groups=[[0, 1, 2, 3], [4, 5, 6, 7]],
                            ins=[rs_in[r][:].opt()],
                            outs=[rs_out[r][:].opt()],
                        ))

                    # ---- schedule: q-chunks processed in order (1,2,3,0) so
                    # the LAST attention chunk is the cheapest (j=0, ~4x less
                    # exp than j=3) -> minimal ACT-paced tail before the final
                    # RS; pairs interleaved per chunk; projections and
                    # out-proj partials drain through the filler queue ----
                    PORDER = (0, 1, 3, 2)

                    def qi(pair, j):
                        if (pair, j) == (0, 0):
                            lo(proj_qk_item(0, 0, wq_sb, qT_sb, 0, QC // 2), 0)
                            lo(proj_qk_item(0, 0, wq_sb, qT_sb,
                                            QC // 2, QC // 2), 0)
                        else:
                            lo(proj_qk_item(pair, j, wq_sb, qT_sb),
                               PORDER.index(j))
                    def ki(pair, j):
                        lo(proj_qk_item(pair, j, wk_sb, kT_sb), PORDER.index(j))
                    def vi(pair, t0, t1, step):
                        for t in range(t0, t1):
                            lo(proj_v_item(pair, t), step)

                    def lo(item, step):
                        fill_lo.append((item[0], item[1], step))

                    marks_pre = {}
                    drains = {}
                    for pair in (0, 1):
                        # step 0 (j=0): everything needed up front
                        qi(pair, 0); ki(pair, 0); vi(pair, 0, 4, 0)
                        marks_pre[(pair, 0)] = len(fill_lo)
                        drains[(pair, 0)] = []
                    for pair in (0, 1):
                        # step 1 (j=1)
                        qi(pair, 1)
                        marks_pre[(pair, 1)] = len(fill_lo)
                        ki(pair, 1); vi(pair, 4, 8, 1)
                        drains[(pair, 1)] = [(0, len(fill_lo))]
                    for pair in (0, 1):
                        # step 2 (j=3): needs k-tiles 2 and 3 plus v 8..15
                        qi(pair, 3)
                        marks_pre[(pair, 3)] = len(fill_lo)
                        ki(pair, 2); vi(pair, 8, 12, 2)
                        m1 = len(fill_lo)
                        ki(pair, 3); vi(pair, 12, 16, 2)
                        drains[(pair, 3)] = [(2, m1), (4, len(fill_lo))]
                    for pair in (0, 1):
                        # step 3 (j=2): only its q projection is new
                        qi(pair, 2)
                        marks_pre[(pair, 2)] = len(fill_lo)
                        drains[(pair, 2)] = []
                    for step, j in enumerate(PORDER):
                        for pair in (0, 1):
                            drain_lo(marks_pre[(pair, j)])
                            attn2(pair, j, drains=drains[(pair, j)], step=step)
                        po_items(j)
                    drain_hi()
                    drain_lo(len(fill_lo))
                    rs(2)
                    # piece readbacks last: their sem waits must not head-of-
                    # line block the SP queue ahead of the scatter DMAs
                    for r in range(3):
                        _rds.append((nc.sync.dma_start(po_out[r][:], rs_out[r][:]), r))

    upd = _ccs[0].ins.sync_info.on_update[0]
    cc_done_sem = bass.SemaphoreHandle(upd.ant_name, upd.id)
    for rd, h in _rds:
        rd.wait_op(cc_done_sem, h + 1, "sem-ge", check=False)
    nc.compile()
    return nc


def _causal_mask():
    # msk[kp, m, qf] = 1 where (m*128 + kp) <= qf else 0  (keep k <= q)
    kp = np.arange(128)[:, None, None]
    m = np.arange(4)[None, :, None]
    qf = np.arange(QC)[None, None, :]
    return (m * 128 + kp <= qf).astype(ml_dtypes.bfloat16)


def _in_maps(x, Wq, Wk, Wv, Wo, bo):
    bf = ml_dtypes.bfloat16
    msk = _causal_mask()
    selv = np.zeros((1, 2, 128), dtype=bf)
    selv[0, 0, 0:64] = 1.0
    selv[0, 1, 64:128] = 1.0
    boq = (bo.reshape(OCT, 128).T / GROUP).astype(np.float32)[:, :, None]
    xT = [np.ascontiguousarray(x[b].T).astype(bf) for b in range(B)]
    maps = []
    for c in range(NCORES):
        b, g = c // GROUP, c % GROUP
        cs = slice(g * CW, (g + 1) * CW)
        maps.append({
            "xT": xT[b],
            "wq": np.ascontiguousarray(Wq[:, cs]).astype(bf),
            "wk": np.ascontiguousarray(Wk[:, cs]).astype(bf),
            "wv": np.ascontiguousarray(Wv[:, cs]).astype(bf),
            "wo": np.ascontiguousarray(Wo[cs, :]).astype(bf),
            "boq": boq,
            "msk": msk,
            "vones": np.ones((128, NKC, HPC, 1), dtype=bf),
            "selv": selv,
        })
    return maps


def kernel(x, Wq, Wk, Wv, Wo, bo, _trace=False):
    x = np.asarray(x, dtype=np.float32)
    Wq, Wk, Wv, Wo, bo = (np.asarray(a, dtype=np.float32) for a in (Wq, Wk, Wv, Wo, bo))
    if "nc" not in _CACHE:
        _CACHE["nc"] = _build_bass()
    nc = _CACHE["nc"]
    res = run_bass_kernel_spmd(
        nc, _in_maps(x, Wq, Wk, Wv, Wo, bo), list(range(NCORES)), trace=_trace
    )
    out = np.zeros((B, S, D), dtype=np.float32)
    for c in range(NCORES):
        b, g = c // GROUP, c % GROUP
        for r in range(3):
            piece = np.asarray(res.results[c][f"po{r}"]).astype(np.float32)
            q0 = RQ0[r] + g * PW[r]
            out[b, q0:q0 + PW[r], :] = piece.T
    if _trace:
        return out, res
    return out


# revision 42
# speedup vs baseline: 1.1064x; 1.0333x over previous
"""Distributed causal multi-head attention for Trainium2 (8 NeuronCores).

Problem (hardcoded): x[2, 2048, 1024], 16 heads, head_dim 64, causal
softmax(QK^T/8)V then out-proj with bias. f32 in/out.

Sharding: data parallel on batch (cores 0-3 -> batch 0, 4-7 -> batch 1),
tensor parallel on heads within each group of 4 (4 heads per core).

Each core:
  - computes Q^T,K^T (head pairs packed to 128 partitions), V for its 4 heads
  - scores transposed S^T[k,q] = K Q^T so the softmax denominator comes out
    of the PE via an appended ones-column on V (no partition reductions)
  - exp without max-subtraction (scores are O(2), safe in fp32/bf16)
  - causal mask applied post-exp as a 0/1 bf16 multiply (DVE 4x mode);
    diagonal k-chunk pairs computed at reduced q-extent (512/256) to cut
    PE scores/ctx and ACT exp work ~12%
  - ROW-parallel out-proj: each core computes partial out[1024 oc, q] over
    its OWN 256 ctx rows (+ bo/4), then a ReduceScatter(add) over the
    4-core group scatters q-slices. Host reassembles q-slices per batch.
    This replaces the ctx AllGather: the collective is billed on its
    (4x smaller) output and RS0 overlaps pair-1 attention.

All matmuls bf16 (fp32 PSUM accumulation).
"""

import numpy as np
import ml_dtypes

from concourse import bass, bacc, mybir
from concourse import tile
from concourse.bass_utils import run_bass_kernel_spmd

BF16 = mybir.dt.bfloat16
F32 = mybir.dt.float32
Act = mybir.ActivationFunctionType

B, S, D = 2, 2048, 1024
H, HD = 16, 64
NCORES = 8
GROUP = 4            # cores per batch group
HPC = H // GROUP     # 4 heads per core
CW = HPC * HD        # 256 ctx rows per core
QC = 512             # q-chunk width
KC = 128             # k-chunk width
NQ = S // QC         # 4
NKC = S // KC        # 16
DCH = D // 128       # 8 contraction chunks of 128
OCT = D // 128       # 8 out-proj column tiles
# RS regions (ascending processing): region 0 = {j0,j1} pieces 256 at
# q=rank*256; region 1 = {j2} pieces 128 at 1024+rank*128; region 2 =
# {j3} pieces 128 at 1536+rank*128 (the tail)
PW = (256, 128, 128)
RQ0 = (0, 1024, 1536)

_CACHE = {}


def _build_bass():
    nc = bacc.Bacc(
        "TRN2", target_bir_lowering=False, debug=False, num_devices=NCORES
    )
    _ccs = []
    _rds = []

    # per-core external inputs (same shapes on every core: SPMD)
    xT = nc.declare_dram_parameter("xT", [D, S], BF16, isOutput=False)
    wq = nc.declare_dram_parameter("wq", [D, CW], BF16, isOutput=False)
    wk = nc.declare_dram_parameter("wk", [D, CW], BF16, isOutput=False)
    wv = nc.declare_dram_parameter("wv", [D, CW], BF16, isOutput=False)
    wo = nc.declare_dram_parameter("wo", [CW, D], BF16, isOutput=False)
    boq = nc.declare_dram_parameter("boq", [128, OCT, 1], F32, isOutput=False)
    msk = nc.declare_dram_parameter("msk", [128, 4, QC], BF16, isOutput=False)
    vones = nc.declare_dram_parameter("vones", [128, NKC, HPC, 1], BF16, isOutput=False)
    selv = nc.declare_dram_parameter("selv", [1, 2, 128], BF16, isOutput=False)
    # ReduceScatter piece outputs: region r piece = q QOFF[r]+rank*PW[r]+
    po_out = [nc.declare_dram_parameter(f"po{h}", [D, w], BF16, isOutput=True)
              for h, w in enumerate(PW)]

    with tile.TileContext(nc) as tc:
        with tc.tile_pool(name="dram", bufs=1, space="DRAM") as dram:
            rs_in = [dram.tile([GROUP, D, w], BF16, name=f"rs_in{h}")
                     for h, w in enumerate(PW)]
            rs_out = [dram.tile([D, w], BF16, name=f"rs_out{h}")
                      for h, w in enumerate(PW)]

            with tc.tile_pool(name="persist", bufs=1) as pp:
                wq_sb = pp.tile([128, DCH, CW], BF16, tag="wq_sb")
                wk_sb = pp.tile([128, DCH, CW], BF16, tag="wk_sb")
                wv_sb = pp.tile([128, DCH, CW], BF16, tag="wv_sb")
                wo_sb = pp.tile([128, CW // 128, D], BF16, tag="wo_sb")
                boq_sb = pp.tile([128, OCT, 1], F32, tag="boq_sb")
                msk_sb = pp.tile([128, 4, QC], BF16, tag="msk_sb")
                selv_sb = pp.tile([65, 2, 128], BF16, tag="selv_sb")
                qT_sb = pp.tile([128, 2, S], BF16, tag="qT_sb")
                kT_sb = pp.tile([128, 2, S], BF16, tag="kT_sb")
                v_aug = pp.tile([128, NKC, HPC, HD + 1], BF16, tag="v_aug")
                xT_sb = pp.tile([128, DCH, S], BF16, tag="xT_sb")

                # ---- input DMAs: few, large, ordered for earliest compute ----
                def ld(dst_ap, src_ap):
                    nc.sync.dma_start(dst_ap, src_ap)

                ld(wq_sb[:], wq.rearrange("(c p) w -> p c w", p=128))
                ld(xT_sb[:, :, 0:QC // 2],
                   xT[:, 0:QC // 2].rearrange("(c p) q -> p c q", p=128))
                ld(wk_sb[:], wk.rearrange("(c p) w -> p c w", p=128))
                ld(xT_sb[:, :, QC // 2:QC],
                   xT[:, QC // 2:QC].rearrange("(c p) q -> p c q", p=128))
                ld(xT_sb[:, :, QC:2 * QC],
                   xT[:, QC:2 * QC].rearrange("(c p) q -> p c q", p=128))
                ld(wv_sb[:], wv.rearrange("(c p) w -> p c w", p=128))
                ld(xT_sb[:, :, 2 * QC:3 * QC],
                   xT[:, 2 * QC:3 * QC].rearrange("(c p) q -> p c q", p=128))
                ld(msk_sb[:], msk[:])
                ld(v_aug[:, :, :, HD:HD + 1], vones[:])
                ld(selv_sb[64:65, :, :], selv[:])
                ld(xT_sb[:, :, 3 * QC:4 * QC],
                   xT[:, 3 * QC:4 * QC].rearrange("(c p) q -> p c q", p=128))
                ld(wo_sb[:], wo.rearrange("(c p) w -> p c w", p=128))
                ld(boq_sb[:], boq[:])

                with tc.tile_pool(name="sc_ps", bufs=2, space="PSUM") as scp, \
                     tc.tile_pool(name="ct_ps", bufs=2, space="PSUM") as ctp, \
                     tc.tile_pool(name="o_ps", bufs=2, space="PSUM") as outp, \
                     tc.tile_pool(name="es_pool", bufs=18) as esp, \
                     tc.tile_pool(name="stg_pool", bufs=8) as stgp, \
                     tc.tile_pool(name="ctxn_pool", bufs=8) as cxp, \
                     tc.tile_pool(name="po_pool", bufs=2) as pop, \
                     tc.tile_pool(name="nrm", bufs=3) as nrmp, \
                     tc.tile_pool(name="misc", bufs=1) as miscp:

                    ctxn = [[None] * NQ, [None] * NQ]

                    # ---- PE filler queue: self-contained items (alloc, use
                    # and release one outp PSUM bank each) popped between
                    # attention chunk iterations so PE never idles while ACT
                    # digests exps, and ACT never idles during projections ----
                    fill_hi = []   # po items: drained first (unblock the RS)
                    fill_lo = []   # proj items: safe to run early
                    _fp = [0, 0]

                    def pop_fill(budget, jmax=NQ):
                        spent = 0.0
                        while spent < budget:
                            if _fp[0] < len(fill_hi):
                                cost, fn = fill_hi[_fp[0]]
                                _fp[0] += 1
                            elif _fp[1] < len(fill_lo) and \
                                    fill_lo[_fp[1]][2] <= jmax:
                                cost, fn = fill_lo[_fp[1]][:2]
                                _fp[1] += 1
                            else:
                                return
                            fn()
                            spent += cost

                    def drain_lo(idx):
                        while _fp[1] < idx:
                            fn = fill_lo[_fp[1]][1]
                            _fp[1] += 1
                            fn()

                    def drain_hi():
                        while _fp[0] < len(fill_hi):
                            cost, fn = fill_hi[_fp[0]]
                            _fp[0] += 1
                            fn()

                    def proj_qk_item(pair, j, w_sb, dst, q0=0, qw=QC):
                        def run():
                            qs = slice(j * QC + q0, j * QC + q0 + qw)
                            ps = outp.tile([128, QC], F32, tag="ops")
                            for c in range(DCH):
                                nc.tensor.matmul(
                                    ps[:, 0:qw],
                                    w_sb[:, c, pair * 128:(pair + 1) * 128],
                                    xT_sb[:, c, qs],
                                    start=(c == 0),
                                    stop=(c == DCH - 1),
                                )
                            nc.vector.tensor_copy(dst[:, pair, qs], ps[:, 0:qw])
                        return (1.9 * qw / QC, run)

                    def proj_v_item(pair, t):
                        def run():
                            ps = outp.tile([128, QC], F32, tag="ops")
                            for c in range(DCH):
                                nc.tensor.matmul(
                                    ps[:, 0:128],
                                    xT_sb[:, c, t * 128:(t + 1) * 128],
                                    wv_sb[:, c, pair * 128:(pair + 1) * 128],
                                    start=(c == 0),
                                    stop=(c == DCH - 1),
                                )
                            nc.vector.tensor_copy(
                                v_aug[:, t, 2 * pair:2 * pair + 2, 0:HD],
                                ps[:, 0:128].rearrange("p (h w) -> p h w", h=2),
                            )
                        return (0.8, run)

                    def attn2(pair, j, drains=(), step=0):
                        # chunk-level software pipeline across the head pair:
                        # PE emits scores(A,i), scores(B,i), then ctx for
                        # chunk-pair i-1 of both heads, so the PE never parks
                        # on the scores->exp->mask chain and ACT stays fed.
                        ha, hb = 2 * pair, 2 * pair + 1
                        nkc = (j + 1) * 4
                        npr = nkc // 2
                        es = {ha: [], hb: []}
                        offs = []

                        def scores_pair(h, i, qa):
                            row = (h % 2) * 64
                            st = scp.tile([128, 2, QC], F32, tag="st")
                            for k in range(2):
                                c = 2 * i + k
                                nc.tensor.matmul(
                                    st[:, k, qa:],
                                    kT_sb[row:row + 64, pair, c * KC:(c + 1) * KC],
                                    qT_sb[row:row + 64, pair, j * QC + qa:(j + 1) * QC],
                                    start=True, stop=True,
                                )
                            e = esp.tile([128, 2, QC], BF16, tag="es")
                            nc.scalar.activation(e[:, :, qa:], st[:, :, qa:],
                                                 Act.Exp, scale=0.125)
                            if i >= npr - 2:
                                m0 = 2 * i - (nkc - 4)
                                nc.vector.tensor_mul(
                                    e[:, :, qa:], e[:, :, qa:],
                                    msk_sb[:, m0:m0 + 2, qa:],
                                )
                            es[h].append(e)

                        def ctx_pair(ct, h, i):
                            qa = offs[i]
                            for k in range(2):
                                c = 2 * i + k
                                nc.tensor.matmul(
                                    ct[:, qa:],
                                    v_aug[:, c, h, :],
                                    es[h][i][:, k, qa:],
                                    start=(c == 0),
                                    stop=(c == nkc - 1),
                                )

                        ctA = ctp.tile([HD + 1, QC], F32, tag="ct")
                        ctB = ctp.tile([HD + 1, QC], F32, tag="ct")
                        for i in range(npr):
                            qa = 256 if i == npr - 1 else 0
                            offs.append(qa)
                            scores_pair(ha, i, qa)
                            scores_pair(hb, i, qa)
                            if i >= 2:
                                ctx_pair(ctA, ha, i - 2)
                                ctx_pair(ctB, hb, i - 2)
                            for di, dmark in drains:
                                if i == di:
                                    drain_lo(dmark)
                            pop_fill(0.8, step + 1)
                        ctx_pair(ctA, ha, npr - 2)
                        ctx_pair(ctB, hb, npr - 2)
                        ctx_pair(ctA, ha, npr - 1)
                        ctx_pair(ctB, hb, npr - 1)
                        # ctx rows + den row in one copy (bf16 den: ~0.2% noise)
                        sa = stgp.tile([HD + 1, QC], BF16, tag="stg")
                        nc.vector.tensor_copy(sa[:], ctA[:])
                        sb = stgp.tile([HD + 1, QC], BF16, tag="stg")
                        nc.vector.tensor_copy(sb[:], ctB[:])
                        norm(pair, j, sa, sb)

                    def norm(pair, j, stg_even, stg_odd):
                        bc = outp.tile([128, QC], F32, tag="ops")
                        nc.tensor.matmul(bc[:], selv_sb[64:65, 0, :],
                                         stg_even[HD:HD + 1, :],
                                         start=True, stop=False)
                        nc.tensor.matmul(bc[:], selv_sb[64:65, 1, :],
                                         stg_odd[HD:HD + 1, :],
                                         start=False, stop=True)
                        rb = nrmp.tile([64, 2, QC], BF16, tag="rb")
                        with nc.allow_low_precision(reason="1/den in bf16: ~0.2% fro"):
                            nc.vector.reciprocal(rb[:, 0, :], bc[0:64, :])
                            nc.vector.reciprocal(rb[:, 1, :], bc[64:128, :])
                        cx = cxp.tile([128, QC], BF16, tag="ctxn")
                        nc.vector.tensor_mul(cx[0:64, :], stg_even[0:HD, :], rb[:, 0, :])
                        nc.vector.tensor_mul(cx[64:128, :], stg_odd[0:HD, :], rb[:, 1, :])
                        ctxn[pair][j] = cx

                    def po_items(j):
                        posb = pop.tile([128, OCT, QC], BF16, tag="posb")
                        tail = (j == 3)

                        def mk_o(o):
                            def run():
                                ps = outp.tile([128, QC], F32, tag="ops")
                                nc.tensor.matmul(
                                    ps[:], wo_sb[:, 0, o * 128:(o + 1) * 128],
                                    ctxn[0][j][:], start=True, stop=False)
                                nc.tensor.matmul(
                                    ps[:], wo_sb[:, 1, o * 128:(o + 1) * 128],
                                    ctxn[1][j][:], start=False, stop=True)
                                with nc.allow_low_precision(reason="bf16 RS partials"):
                                    if tail and o % 2 == 1:
                                        # ACT is idle in the tail: alternate so
                                        # the po stream isn't DVE-paced
                                        nc.scalar.activation(
                                            posb[:, o, :], ps[:], Act.Identity,
                                            bias=boq_sb[:, o, :])
                                    else:
                                        nc.vector.tensor_scalar_add(
                                            posb[:, o, :], ps[:], boq_sb[:, o, :])
                                if j in (2, 3):
                                    # 128-wide-piece regions: per-o-tile scatter
                                    # fires immediately so the RS input is
                                    # ready as soon as the last add lands
                                    r = 1 if j == 2 else 2
                                    nc.sync.dma_start(
                                        rs_in[r][:, o * 128:(o + 1) * 128, :]
                                        .rearrange("r p q -> p r q"),
                                        posb[:, o, :].rearrange(
                                            "p (r q) -> p r q", r=GROUP),
                                    )
                            return (0.6, run)

                        def dma_run():
                            # region 0 ({j0,j1}, 256-wide pieces): j-chunks
                            # align to exactly two slabs
                            for s2 in range(2):
                                slab = 2 * j + s2
                                nc.sync.dma_start(
                                    rs_in[0][slab, :, :].rearrange(
                                        "(o p) q -> p o q", p=128),
                                    posb[:, :, s2 * 256:(s2 + 1) * 256],
                                )

                        for o in range(OCT):
                            fill_hi.append(mk_o(o))
                        if j in (0, 1):
                            fill_hi.append((0.1, dma_run))
                        if j == 1:
                            fill_hi.append((0.1, lambda: rs(0)))
                        if j == 2:
                            fill_hi.append((0.1, lambda: rs(1)))

                    def rs(r):
                        _ccs.append(nc.gpsimd.collective_compute(
                            "ReduceScatter",
                            mybir.AluOpType.add,
                            replica_groups=[[0, 1, 2, 3], [4, 5, 6, 7]],
                            ins=[rs_in[r][:].opt()],
                            outs=[rs_out[r][:].opt()],
                        ))

                    # ---- schedule: q-chunks processed in order (1,2,3,0) so
                    # the LAST attention chunk is the cheapest (j=0, ~4x less
                    # exp than j=3) -> minimal ACT-paced tail before the final
                    # RS; pairs interleaved per chunk; projections and
                    # out-proj partials drain through the filler queue ----
                    PORDER = (0, 1, 2, 3)

                    def qi(pair, j):
                        if (pair, j) == (0, 0):
                            lo(proj_qk_item(0, 0, wq_sb, qT_sb, 0, QC // 2), 0)
                            lo(proj_qk_item(0, 0, wq_sb, qT_sb,
                                            QC // 2, QC // 2), 0)
                        else:
                            lo(proj_qk_item(pair, j, wq_sb, qT_sb),
                               PORDER.index(j))
                    def ki(pair, j):
                        lo(proj_qk_item(pair, j, wk_sb, kT_sb), PORDER.index(j))
                    def vi(pair, t0, t1, step):
                        for t in range(t0, t1):
                            lo(proj_v_item(pair, t), step)

                    def lo(item, step):
                        fill_lo.append((item[0], item[1], step))

                    marks_pre = {}
                    drains = {}
                    for pair in (0, 1):
                        # step 0 (j=0): everything needed up front
                        qi(pair, 0); ki(pair, 0); vi(pair, 0, 4, 0)
                        marks_pre[(pair, 0)] = len(fill_lo)
                        drains[(pair, 0)] = []
                    for pair in (0, 1):
                        # step 1 (j=1)
                        qi(pair, 1)
                        marks_pre[(pair, 1)] = len(fill_lo)
                        ki(pair, 1); vi(pair, 4, 8, 1)
                        drains[(pair, 1)] = [(0, len(fill_lo))]
                    for pair in (0, 1):
                        # step 2 (j=2)
                        qi(pair, 2)
                        marks_pre[(pair, 2)] = len(fill_lo)
                        ki(pair, 2); vi(pair, 8, 12, 2)
                        drains[(pair, 2)] = [(2, len(fill_lo))]
                    for pair in (0, 1):
                        # step 3 (j=3)
                        qi(pair, 3)
                        marks_pre[(pair, 3)] = len(fill_lo)
                        ki(pair, 3); vi(pair, 12, 16, 3)
                        drains[(pair, 3)] = [(4, len(fill_lo))]
                    for step, j in enumerate(PORDER):
                        for pair in (0, 1):
                            drain_lo(marks_pre[(pair, j)])
                            attn2(pair, j, drains=drains[(pair, j)], step=step)
                        po_items(j)
                    drain_hi()
                    drain_lo(len(fill_lo))
                    rs(2)
                    # piece readbacks last: their sem waits must not head-of-
                    # line block the SP queue ahead of the scatter DMAs
                    for r in range(3):
                        _rds.append((nc.sync.dma_start(po_out[r][:], rs_out[r][:]), r))

    upd = _ccs[0].ins.sync_info.on_update[0]
    cc_done_sem = bass.SemaphoreHandle(upd.ant_name, upd.id)
    for rd, h in _rds:
        rd.wait_op(cc_done_sem, h + 1, "sem-ge", check=False)
    nc.compile()
    return nc


def _causal_mask():
    # msk[kp, m, qf] = 1 where (m*128 + kp) <= qf else 0  (keep k <= q)
    kp = np.arange(128)[:, None, None]
    m = np.arange(4)[None, :, None]
    qf = np.arange(QC)[None, None, :]
    return (m * 128 + kp <= qf).astype(ml_dtypes.bfloat16)


def _in_maps(x, Wq, Wk, Wv, Wo, bo):
    bf = ml_dtypes.bfloat16
    msk = _causal_mask()
    selv = np.zeros((1, 2, 128), dtype=bf)
    selv[0, 0, 0:64] = 1.0
    selv[0, 1, 64:128] = 1.0
    boq = (bo.reshape(OCT, 128).T / GROUP).astype(np.float32)[:, :, None]
    xT = [np.ascontiguousarray(x[b].T).astype(bf) for b in range(B)]
    maps = []
    for c in range(NCORES):
        b, g = c // GROUP, c % GROUP
        cs = slice(g * CW, (g + 1) * CW)
        maps.append({
            "xT": xT[b],
            "wq": np.ascontiguousarray(Wq[:, cs]).astype(bf),
            "wk": np.ascontiguousarray(Wk[:, cs]).astype(bf),
            "wv": np.ascontiguousarray(Wv[:, cs]).astype(bf),
            "wo": np.ascontiguousarray(Wo[cs, :]).astype(bf),
            "boq": boq,
            "msk": msk,
            "vones": np.ones((128, NKC, HPC, 1), dtype=bf),
            "selv": selv,
        })
    return maps


def kernel(x, Wq, Wk, Wv, Wo, bo, _trace=False):
    x = np.asarray(x, dtype=np.float32)
    Wq, Wk, Wv, Wo, bo = (np.asarray(a, dtype=np.float32) for a in (Wq, Wk, Wv, Wo, bo))
    if "nc" not in _CACHE:
        _CACHE["nc"] = _build_bass()
    nc = _CACHE["nc"]
    res = run_bass_kernel_spmd(
        nc, _in_maps(x, Wq, Wk, Wv, Wo, bo), list(range(NCORES)), trace=_trace
    )
    out = np.zeros((B, S, D), dtype=np.float32)
    for c in range(NCORES):
        b, g = c // GROUP, c % GROUP
        for r in range(3):
            piece = np.asarray(res.results[c][f"po{r}"]).astype(np.float32)
            q0 = RQ0[r] + g * PW[r]
            out[b, q0:q0 + PW[r], :] = piece.T
    if _trace:
        return out, res
    return out
